# revision 16
# baseline (speedup 1.0000x reference)
"""Trainium2 Bass kernel for DigitConvolutionalModel (dense_cnn).

Network: x[B,784] -> 3x3 valid conv (1 channel) -> flatten[676] -> 4-layer MLP
         (676->200 relu, 200->200 relu, 200->200 relu, 200->10).

Key algebraic fold: the conv is linear and feeds the first dense layer with
no nonlinearity in between (reshape only), so conv+W1 collapse into a single
effective weight W1p = C @ W1 of shape [784, 200], where C is the im2col
matrix of conv_w. The whole network becomes a 4-layer MLP:

    out = relu(relu(relu(x @ W1p + b1) @ W2 + b2) @ W3 + b3) @ W4 + b4

Sharding: pure data parallel over 8 cores (batch 65536 -> 8192/core).
On-device dataflow keeps activations feature-major ([features, batch]) so
every layer is a plain lhsT.T @ rhs matmul chain with no on-chip transposes.
The host pre-arranges each core's x shard as [NG*112, 7*512] so each batch
group of 512 samples loads with ONE dma_start whose per-partition runs are
fully contiguous (14 KB), and packs all weights+biases into one [128, WC]
blob loaded by one DMA. Outputs leave as [10, 8192] per core and the host
transposes them back.

Matmul dtype mode: "f32r" streams the PE at 1 cycle/row (vs 4 for full
fp32) with ~tf32 precision; measured end-to-end relative error ~4e-4.
PE-visible tensors (x, weights, hidden activations) are declared float32r;
the fp32 bit layout is unchanged so host data passes through as-is.
"""

import numpy as np

import concourse.bacc as bacc
import concourse.mybir as mybir
import concourse.tile as tile
from concourse.bass_utils import run_bass_kernel_spmd

B = 65536
IMG = 28
KW = 3
CONV_OUT = (IMG - KW + 1) ** 2  # 676
HID = 200
OUT = 10
K1 = IMG * IMG  # 784

N_CORES = 8
BC = B // N_CORES  # 8192 rows per core
NB = 512  # batch columns per matmul group (one PSUM bank)
NG = BC // NB  # 16 groups
K1C = 112  # K1 split into 7 chunks of 112 (<=128)
NK1 = K1 // K1C  # 7
# 200 split for partition-dim chunks (both as matmul M and as next-layer K)
HCHUNKS = [(0, 128), (128, 72)]

F32 = mybir.dt.float32

# ---- weight-blob column layout (shared by host packer + device slicer) ----
W1_COL = 0                      # 7 chunks of [112, 200]
W2_COL = W1_COL + NK1 * HID     # 2 chunks of [128/72, 200]
W3_COL = W2_COL + 2 * HID
W4_COL = W3_COL + 2 * HID       # 2 chunks of [128/72, 10]
B_COL = W4_COL + 2 * OUT        # b1(2) b2(2) b3(2) cols, then b4
B4_COL = B_COL + 6
W4P_COL = B4_COL + 1            # 2 chunks of [128/72, 32]: W4 zero-padded to M=32
WC = W4P_COL + 2 * 32

_cache: dict = {}

# ---- bf16 weight-blob column layout (element indices, bf16) ----
BW1_COL = 0                       # 7 chunks of [112, 200]
BW2_COL = BW1_COL + NK1 * HID     # 2 chunks of [128/72, 200]
BW3_COL = BW2_COL + 2 * HID
BW4_COL = BW3_COL + 2 * HID       # 2 chunks of [128/72, 10]
BWC = BW4_COL + 2 * OUT
# bias tensor [128, 7] f32: cols 0-1 b1, 2-3 b2, 4-5 b3, 6 b4


def _build_bf16(repeats: int = 1, xbufs: int = 3, hbufs: int = 2,
                obufs: int = 2, skip_l23: bool = False, skew: bool = True,
                dummy_h: bool = False):
    """All-bf16 variant: x, weights, and hidden activations in bfloat16
    (PSUM accumulation stays fp32; biases applied from a separate fp32
    tensor). Measured on this TRN2 terminal, bf16 moving operands stream
    substantially faster than f32r and input HBM traffic halves."""
    BF = mybir.dt.bfloat16

    nc = bacc.Bacc("TRN2", target_bir_lowering=False, debug=False)

    xh = nc.dram_tensor("xh", [NG * K1C, NK1 * NB], BF, kind="ExternalInput")
    wb = nc.dram_tensor("wb", [128, BWC], BF, kind="ExternalInput")
    bias = nc.dram_tensor("bias", [128, 7], F32, kind="ExternalInput")
    outT = nc.dram_tensor("outT", [OUT, BC], F32, kind="ExternalOutput")

    relu = mybir.ActivationFunctionType.Relu

    with tile.TileContext(nc) as tc:
        with (
            tc.tile_pool(name="wpool", bufs=1) as wpool,
            tc.tile_pool(name="xpool", bufs=xbufs) as xpool,
            tc.tile_pool(name="hpool", bufs=hbufs) as hpool,
            tc.tile_pool(name="opool", bufs=obufs) as opool,
            tc.tile_pool(name="psum", bufs=1, space="PSUM") as psum,
        ):
            wt = wpool.tile([128, BWC], BF, tag="wt")
            bt = wpool.tile([128, 7], F32, tag="bt")
            nc.scalar.dma_start(out=wt[:, 0:BW2_COL], in_=wb.ap()[:, 0:BW2_COL])
            nc.gpsimd.dma_start(out=wt[:, BW2_COL:BWC], in_=wb.ap()[:, BW2_COL:BWC])
            nc.gpsimd.dma_start(out=bt, in_=bias.ap())

            def w1s(k, h0, hsz):
                return wt[0:K1C, BW1_COL + k * HID + h0 : BW1_COL + k * HID + h0 + hsz]

            def w23s(base, k, h0, hsz):
                k0, ksz = HCHUNKS[k]
                c = base + k * HID + h0
                return wt[0:ksz, c : c + hsz]

            def w4s(k):
                k0, ksz = HCHUNKS[k]
                c = BW4_COL + k * OUT
                return wt[0:ksz, c : c + OUT]

            def bs(idx, hsz):
                return bt[0:hsz, idx : idx + 1]

            def relu_evac(h, ps, bias_ap, eng):
                # PSUM evacuation split across the two engines with PSUM
                # read ports (different banks proceed in parallel):
                # ACT handles L1+L3 chunks, DVE handles L2 chunks + L4.
                if eng == "act":
                    nc.scalar.activation(h, ps, relu, bias=bias_ap)
                else:
                    nc.vector.tensor_scalar(
                        h, ps, bias_ap, 0.0,
                        op0=mybir.AluOpType.add, op1=mybir.AluOpType.max,
                    )

            if dummy_h:
                hd0 = wpool.tile([128, NB], BF, tag="hd0")
                hd1 = wpool.tile([128, NB], BF, tag="hd1")
                nc.vector.memset(hd0, 0.25)
                nc.vector.memset(hd1, 0.25)

            def l1_chunk(xg, i):
                h0, hsz = HCHUNKS[i]
                ps = psum.tile([hsz, NB], F32, tag=f"ps1_{i}")
                for k in range(NK1):
                    nc.tensor.matmul(
                        ps, w1s(k, h0, hsz), xg[:, k, :],
                        start=(k == 0), stop=(k == NK1 - 1),
                    )
                if dummy_h:
                    return (hd0 if i == 0 else hd1)[0:hsz, :]
                h = hpool.tile([hsz, NB], BF, tag=f"h1_{i}")
                relu_evac(h, ps, bs(i, hsz), "act")
                return h

            def dense(hin, base, li):
                # k-major emission: both M-chunks' k=0 matmuls first, so the
                # PE has extra cover while hin[1]'s evacuation finishes.
                ps = [
                    psum.tile([hsz, NB], F32, tag=f"ps{li}_{i}",
                              name=f"ps{li}_{i}")
                    for i, (h0, hsz) in enumerate(HCHUNKS)
                ]
                nk = len(HCHUNKS)
                for k in range(nk):
                    for i, (h0, hsz) in enumerate(HCHUNKS):
                        nc.tensor.matmul(
                            ps[i], w23s(base, k, h0, hsz), hin[k],
                            start=(k == 0), stop=(k == nk - 1),
                            skip_group_check=True,
                        )
                if dummy_h:
                    return [hd0[0:128, :], hd1[0:72, :]]
                hout = []
                for i, (h0, hsz) in enumerate(HCHUNKS):
                    h = hpool.tile([hsz, NB], BF, tag=f"h{li}_{i}")
                    relu_evac(h, ps[i], bs(2 * (li - 1) + i, hsz),
                              "act" if li == 3 else "dve")
                    hout.append(h)
                return hout

            def l4_out(hin, g):
                ps = psum.tile([OUT, NB], F32, tag="ps4")
                for k in range(len(HCHUNKS)):
                    nc.tensor.matmul(
                        ps, w4s(k), hin[k],
                        start=(k == 0), stop=(k == len(HCHUNKS) - 1),
                    )
                o = opool.tile([OUT, NB], F32, tag="o")
                nc.vector.tensor_scalar_add(o, ps, bt[0:OUT, 6:7])
                # Trigger the output DMA via SWDGE (idle after the weight
                # load) — a scalar-queue trigger would sit in ACT's strict
                # FIFO waiting on the L4 chain and stall later evacuations.
                nc.gpsimd.dma_start(out=outT.ap()[:, g * NB : (g + 1) * NB], in_=o)

            def load_x(t):
                g = t % NG
                xg = xpool.tile([K1C, NK1, NB], BF, tag="xg")
                src = xh.ap()[g * K1C : (g + 1) * K1C, :].rearrange(
                    "p (k b) -> p k b", k=NK1
                )
                if t == 0:
                    for k in range(NK1):
                        nc.sync.dma_start(out=xg[:, k, :], in_=src[:, k, :])
                else:
                    nc.sync.dma_start(out=xg, in_=src)
                return xg

            NT = NG * repeats
            if skew:
                # Fully-skewed pipeline: iteration t runs L1(t), L2(t-1),
                # L3(t-2), L4(t-3) — every consumer is a full iteration
                # behind its producer, so no PSUM evacuation is ever on the
                # PE's critical path.
                h1d, h2d, h3d = {}, {}, {}
                for t in range(NT + 3):
                    if t < NT:
                        xg = load_x(t)
                        h1d[t] = [l1_chunk(xg, 0)]
                    if t - 1 >= 0 and t - 1 < NT:
                        if skip_l23:
                            h2d[t - 1] = h1d[t - 1]
                        else:
                            h2d[t - 1] = dense(h1d[t - 1], BW2_COL, 2)
                    if t < NT:
                        h1d[t].append(l1_chunk(xg, 1))
                    if t - 2 >= 0 and t - 2 < NT:
                        if skip_l23:
                            h3d[t - 2] = h2d.pop(t - 2)
                        else:
                            h3d[t - 2] = dense(h2d.pop(t - 2), BW3_COL, 3)
                    if t - 3 >= 0 and t - 3 < NT:
                        l4_out(h3d.pop(t - 3), (t - 3) % NG)
                    h1d.pop(t - 1, None)
            else:
                h2_prev = None
                prev_g = None
                for t in range(NT):
                    g = t % NG
                    xg = load_x(t)
                    h1_0 = l1_chunk(xg, 0)
                    if skip_l23:
                        h3_prev = h2_prev
                    else:
                        h3_prev = (dense(h2_prev, BW3_COL, 3)
                                   if h2_prev is not None else None)
                    h1_1 = l1_chunk(xg, 1)
                    if h3_prev is not None:
                        l4_out(h3_prev, prev_g)
                    if skip_l23:
                        h2_prev = [h1_0, h1_1]
                    else:
                        h2_prev = dense([h1_0, h1_1], BW2_COL, 2)
                    prev_g = g
                if skip_l23:
                    l4_out(h2_prev, prev_g)
                else:
                    l4_out(dense(h2_prev, BW3_COL, 3), prev_g)

    nc.compile()
    return nc


def _build(mode: str, repeats: int = 1, xbufs: int = 3, hbufs: int = 2,
           obufs: int = 2, pack_l4: bool = False, x_bf16: bool = False,
           skip_l23: bool = False):
    """Build + compile the per-core Bass program (same NEFF on all cores).

    repeats>1 re-runs the whole batch loop (same data) inside one NEFF —
    used only for benchmarking device time by slope.

    pack_l4: run L4's M=10 matmuls in PE column group 3 (tile_position
    (0, 96)) concurrently with the last two L1m1 (M=72, col groups 0-2)
    matmuls — would hide L4's 2 matmul-times (~7 us/pass). DO NOT USE:
    this neuronxcc build rejects any matmul dst PSUM at non-zero base
    partition (ISA check 's3d3_mm_valid_dst_partition'), even 32-aligned
    full-strip slices. Kept for documentation.
    """
    DT = mybir.dt.float32r if mode == "f32r" else F32
    XDT = mybir.dt.bfloat16 if x_bf16 else DT

    nc = bacc.Bacc("TRN2", target_bir_lowering=False, debug=False)

    xh = nc.dram_tensor("xh", [NG * K1C, NK1 * NB], XDT, kind="ExternalInput")
    wb = nc.dram_tensor("wb", [128, WC], DT, kind="ExternalInput")
    if x_bf16:
        wb1 = nc.dram_tensor(
            "wb1", [K1C, NK1 * HID], mybir.dt.bfloat16, kind="ExternalInput"
        )
    outT = nc.dram_tensor("outT", [OUT, BC], F32, kind="ExternalOutput")

    relu = mybir.ActivationFunctionType.Relu

    with tile.TileContext(nc) as tc:
        with (
            tc.tile_pool(name="wpool", bufs=1) as wpool,
            tc.tile_pool(name="xpool", bufs=xbufs) as xpool,
            tc.tile_pool(name="hpool", bufs=hbufs) as hpool,
            tc.tile_pool(name="opool", bufs=obufs) as opool,
            tc.tile_pool(name="psum", bufs=1, space="PSUM") as psum,
        ):
            # Weight load split across rings so it overlaps the first x-group
            # load: w1p columns (needed first) on the ACT HWDGE ring, the
            # rest (needed only from L2 on) via SWDGE. The SP HWDGE ring
            # stays a pure back-to-back stream of x-group loads.
            wt = wpool.tile([128, WC], DT, tag="wt")
            if x_bf16:
                wt1 = wpool.tile([K1C, NK1 * HID], mybir.dt.bfloat16, tag="wt1")
                nc.scalar.dma_start(out=wt1, in_=wb1.ap())
                nc.gpsimd.dma_start(out=wt[:, W2_COL:WC], in_=wb.ap()[:, W2_COL:WC])
            else:
                nc.scalar.dma_start(out=wt[:, 0:W2_COL], in_=wb.ap()[:, 0:W2_COL])
                nc.gpsimd.dma_start(out=wt[:, W2_COL:WC], in_=wb.ap()[:, W2_COL:WC])

            def w1s(k, h0, hsz):  # lhsT [112, hsz] for L1 chunk k
                if x_bf16:
                    return wt1[0:K1C, k * HID + h0 : k * HID + h0 + hsz]
                return wt[0:K1C, W1_COL + k * HID + h0 : W1_COL + k * HID + h0 + hsz]

            def w23s(base, k, h0, hsz):  # lhsT [ksz, hsz] for L2/L3
                k0, ksz = HCHUNKS[k]
                c = base + k * HID + h0
                return wt[0:ksz, c : c + hsz]

            def w4s(k):  # lhsT [ksz, 10]
                k0, ksz = HCHUNKS[k]
                c = W4_COL + k * OUT
                return wt[0:ksz, c : c + OUT]

            def w4ps(k):  # lhsT [ksz, 32] (W4 zero-padded, for col-group 3)
                k0, ksz = HCHUNKS[k]
                c = W4P_COL + k * 32
                return wt[0:ksz, c : c + 32]

            def bs(idx, hsz):  # bias column [hsz, 1] as f32
                return wt[0:hsz, B_COL + idx : B_COL + idx + 1].bitcast(F32)

            # ---- emission helpers ----
            def l1_chunk(xg, i, l4_pair=None):
                h0, hsz = HCHUNKS[i]
                ps = psum.tile([hsz, NB], F32, tag=f"ps1_{i}")
                ps4 = None
                if l4_pair is not None:
                    hin3, g4 = l4_pair
                    ps4full = psum.tile([128, NB], F32, tag="ps4")
                    ps4 = ps4full[96:128, :]
                for k in range(NK1):
                    nc.tensor.matmul(
                        ps, w1s(k, h0, hsz), xg[:, k, :],
                        start=(k == 0), stop=(k == NK1 - 1),
                        skip_group_check=l4_pair is not None,
                    )
                    if l4_pair is not None and k >= NK1 - 2:
                        # L4 (M=10) in col group 3, concurrent with this
                        # M=72 matmul occupying col groups 0-2
                        kk = k - (NK1 - 2)
                        nc.tensor.matmul(
                            ps4, w4ps(kk), l4_pair[0][kk],
                            start=(kk == 0), stop=(kk == 1),
                            tile_position=(0, 96), skip_group_check=True,
                        )
                h = hpool.tile([hsz, NB], DT, tag=f"h1_{i}")
                nc.scalar.activation(h, ps, relu, bias=bs(i, hsz))
                if l4_pair is not None:
                    g4 = l4_pair[1]
                    o = opool.tile([128, NB], F32, tag="o")
                    nc.vector.tensor_scalar_add(
                        o[96 : 96 + OUT, :], ps4[0:OUT, :],
                        wt[96 : 96 + OUT, B4_COL : B4_COL + 1].bitcast(F32),
                    )
                    nc.scalar.dma_start(
                        out=outT.ap()[:, g4 * NB : (g4 + 1) * NB],
                        in_=o[96 : 96 + OUT, :],
                    )
                return h

            def dense(hin, base, li):  # L2 (li=2) / L3 (li=3) full layer
                hout = []
                for i, (h0, hsz) in enumerate(HCHUNKS):
                    ps = psum.tile([hsz, NB], F32, tag=f"ps{li}_{i}")
                    for k in range(len(HCHUNKS)):
                        nc.tensor.matmul(
                            ps, w23s(base, k, h0, hsz), hin[k],
                            start=(k == 0), stop=(k == len(HCHUNKS) - 1),
                        )
                    h = hpool.tile([hsz, NB], DT, tag=f"h{li}_{i}")
                    nc.scalar.activation(
                        h, ps, relu, bias=bs(2 * (li - 1) + i, hsz)
                    )
                    hout.append(h)
                return hout

            def l4_out(hin, g):
                ps = psum.tile([OUT, NB], F32, tag="ps4")
                for k in range(len(HCHUNKS)):
                    nc.tensor.matmul(
                        ps, w4s(k), hin[k],
                        start=(k == 0), stop=(k == len(HCHUNKS) - 1),
                    )
                o = opool.tile([OUT, NB], F32, tag="o")
                nc.vector.tensor_scalar_add(
                    o, ps, wt[0:OUT, B4_COL : B4_COL + 1].bitcast(F32)
                )
                # ACT HWDGE ring: keeps the SP ring a pure back-to-back
                # stream of x-group loads (no head-of-line blocking on the
                # late-produced outputs).
                nc.scalar.dma_start(out=outT.ap()[:, g * NB : (g + 1) * NB], in_=o)

            # ---- main loop: groups software-pipelined with a 1-group skew.
            # PE stream per iteration: L1m0(t) | L3(t-1) | L1m1(t) | L4(t-1)
            # | L2(t) — the independent L1 matmuls hide the ACT latency of
            # the previous group's dependent L3/L4 chain.
            h2_prev = None
            prev_g = None
            for t in range(NG * repeats):
                g = t % NG
                xg = xpool.tile([K1C, NK1, NB], XDT, tag="xg")
                src = xh.ap()[g * K1C : (g + 1) * K1C, :].rearrange(
                    "p (k b) -> p k b", k=NK1
                )
                if t == 0:
                    # per-k-chunk loads: the first matmul starts after one
                    # 224 KB chunk instead of the whole 1.6 MB group
                    for k in range(NK1):
                        nc.sync.dma_start(out=xg[:, k, :], in_=src[:, k, :])
                else:
                    nc.sync.dma_start(out=xg, in_=src)
                h1_0 = l1_chunk(xg, 0)
                if skip_l23:
                    h3_prev = h2_prev
                else:
                    h3_prev = dense(h2_prev, W3_COL, 3) if h2_prev is not None else None
                if pack_l4 and h3_prev is not None:
                    h1_1 = l1_chunk(xg, 1, l4_pair=(h3_prev, prev_g))
                else:
                    h1_1 = l1_chunk(xg, 1)
                    if h3_prev is not None:
                        l4_out(h3_prev, prev_g)
                if skip_l23:
                    h2_prev = [h1_0, h1_1]
                else:
                    h2_prev = dense([h1_0, h1_1], W2_COL, 2)
                prev_g = g
            # epilogue: finish the last group
            if skip_l23:
                l4_out(h2_prev, prev_g)
            else:
                l4_out(dense(h2_prev, W3_COL, 3), prev_g)

    nc.compile()
    return nc


def _im2col(conv_w: np.ndarray) -> np.ndarray:
    """C[784, 676] with h_conv = x @ C (cross-correlation, valid)."""
    co = IMG - KW + 1
    C = np.zeros((IMG * IMG, co * co), dtype=np.float64)
    ii, jj = np.meshgrid(np.arange(co), np.arange(co), indexing="ij")
    q = (ii * co + jj).ravel()
    for di in range(KW):
        for dj in range(KW):
            p = ((ii + di) * IMG + (jj + dj)).ravel()
            C[p, q] += conv_w[di, dj]
    return C


def _pack_weights(W1p, b1, W2, b2, W3, b3, W4, b4) -> np.ndarray:
    wb = np.zeros((128, WC), dtype=np.float32)
    for k in range(NK1):
        wb[0:K1C, W1_COL + k * HID : W1_COL + (k + 1) * HID] = W1p[
            k * K1C : (k + 1) * K1C
        ]
    for i, (h0, hsz) in enumerate(HCHUNKS):
        wb[0:hsz, W2_COL + i * HID : W2_COL + (i + 1) * HID] = W2[h0 : h0 + hsz]
        wb[0:hsz, W3_COL + i * HID : W3_COL + (i + 1) * HID] = W3[h0 : h0 + hsz]
        wb[0:hsz, W4_COL + i * OUT : W4_COL + (i + 1) * OUT] = W4[h0 : h0 + hsz]
        wb[0:hsz, W4P_COL + i * 32 : W4P_COL + i * 32 + OUT] = W4[h0 : h0 + hsz]
        wb[0:hsz, B_COL + i] = b1[h0 : h0 + hsz]
        wb[0:hsz, B_COL + 2 + i] = b2[h0 : h0 + hsz]
        wb[0:hsz, B_COL + 4 + i] = b3[h0 : h0 + hsz]
    wb[0:OUT, B4_COL] = b4
    wb[96 : 96 + OUT, B4_COL] = b4  # copy at partition 96 for the packed-L4 path
    return wb


def _pack_weights_bf16(W1p, W2, W3, W4):
    import ml_dtypes

    wb = np.zeros((128, BWC), dtype=ml_dtypes.bfloat16)
    for k in range(NK1):
        wb[0:K1C, BW1_COL + k * HID : BW1_COL + (k + 1) * HID] = W1p[
            k * K1C : (k + 1) * K1C
        ].astype(ml_dtypes.bfloat16)
    for i, (h0, hsz) in enumerate(HCHUNKS):
        wb[0:hsz, BW2_COL + i * HID : BW2_COL + (i + 1) * HID] = W2[
            h0 : h0 + hsz
        ].astype(ml_dtypes.bfloat16)
        wb[0:hsz, BW3_COL + i * HID : BW3_COL + (i + 1) * HID] = W3[
            h0 : h0 + hsz
        ].astype(ml_dtypes.bfloat16)
        wb[0:hsz, BW4_COL + i * OUT : BW4_COL + (i + 1) * OUT] = W4[
            h0 : h0 + hsz
        ].astype(ml_dtypes.bfloat16)
    return wb


def _pack_bias(b1, b2, b3, b4):
    bias = np.zeros((128, 7), dtype=np.float32)
    for i, (h0, hsz) in enumerate(HCHUNKS):
        bias[0:hsz, 0 + i] = b1[h0 : h0 + hsz]
        bias[0:hsz, 2 + i] = b2[h0 : h0 + hsz]
        bias[0:hsz, 4 + i] = b3[h0 : h0 + hsz]
    bias[0:OUT, 6] = b4
    return bias


def _pack_x(x_shard: np.ndarray, bf16: bool = False) -> np.ndarray:
    """[8192, 784] -> [NG*112, 7*512]: row g*112+p holds, for each k-chunk,
    the 512 batch values of pixel k*112+p in group g (contiguous per row)."""
    # xT[k*112+p, g*512+b] -> xh[g, p, k, b]
    xt = x_shard.T.reshape(NK1, K1C, NG, NB)  # [k, p, g, b]
    xh = np.ascontiguousarray(xt.transpose(2, 1, 0, 3))  # [g, p, k, b]
    xh = xh.reshape(NG * K1C, NK1 * NB)
    if bf16:
        import ml_dtypes

        xh = xh.astype(ml_dtypes.bfloat16)
    return xh


def make_in_maps(mode, x, conv_w, W1, b1, W2, b2, W3, b3, W4, b4):
    x = np.asarray(x, dtype=np.float32)
    C = _im2col(np.asarray(conv_w, dtype=np.float64))
    W1p = (C @ np.asarray(W1, dtype=np.float64)).astype(np.float32)
    if mode == "bf16":
        wb = _pack_weights_bf16(
            W1p, np.asarray(W2, np.float32), np.asarray(W3, np.float32),
            np.asarray(W4, np.float32),
        )
        bias = _pack_bias(
            np.asarray(b1, np.float32), np.asarray(b2, np.float32),
            np.asarray(b3, np.float32), np.asarray(b4, np.float32),
        )
        return [
            {"xh": _pack_x(x[c * BC : (c + 1) * BC], bf16=True), "wb": wb,
             "bias": bias}
            for c in range(N_CORES)
        ]
    wb = _pack_weights(
        W1p,
        np.asarray(b1, np.float32), np.asarray(W2, np.float32),
        np.asarray(b2, np.float32), np.asarray(W3, np.float32),
        np.asarray(b3, np.float32), np.asarray(W4, np.float32),
        np.asarray(b4, np.float32),
    )
    return [
        {"xh": _pack_x(x[c * BC : (c + 1) * BC]), "wb": wb}
        for c in range(N_CORES)
    ]


def build_mode(mode, repeats=1, **kwargs):
    if mode == "bf16":
        return _build_bf16(repeats=repeats, **kwargs)
    return _build(mode, repeats=repeats, **kwargs)


DEFAULT_MODE = "bf16"


def kernel(x, conv_w, W1, b1, W2, b2, W3, b3, W4, b4, _mode=DEFAULT_MODE):
    if _mode not in _cache:
        _cache[_mode] = build_mode(_mode)
    nc = _cache[_mode]

    in_maps = make_in_maps(_mode, x, conv_w, W1, b1, W2, b2, W3, b3, W4, b4)
    res = run_bass_kernel_spmd(nc, in_maps, core_ids=list(range(N_CORES)))

    out = np.empty((B, OUT), dtype=np.float32)
    for c in range(N_CORES):
        out[c * BC : (c + 1) * BC] = res.results[c]["outT"].T
    return out

